# revision 1
# baseline (speedup 1.0000x reference)
"""TRN2 Bass kernel for nn_ClassicalSelfAttention (N=8192, D=1024) on 8 NeuronCores.

Math: out = softmax((X R)(X E)^T / sqrt(D)) X
Row-sharded over 8 cores (m = rows of the query/output). Per core:
    QT = R^T Xi^T              [c, m]   (lhsT=R, rhs=XiT)
    PT = E^T QT                [d, m]   = (Xi R E^T)^T
    LT_j = XT_j^T PT           [n_j, m] logits, transposed layout
    softmax over n (partition axis): per-chunk DVE max/sum accumulation,
    cross-partition finish via PE transpose + DVE reduce, broadcast of
    -max back across partitions via a K=1 ones-matmul, then DVE add +
    ACT exp (scale=1/sqrt(D) folded into the activation's affine).
    out = sum_j E_j^T X_j      (AV; lhsT=exp-scores chunk, rhs=X chunk)

Precision: the whole matmul chain runs in fp16 at full PE rate (1 cyc/row).
The logits chain (QT/PT/LT) uses a 3-matmul fp16 hi/lo split
(hi*hi + hi*lo + lo*hi), giving ~fp32-grade logits (max scaled-logit error
~8e-4). AV uses single fp16 (scores in [0,1], values |x|~1: ~5e-4 relative).
Inputs are split/rounded on the host. Measured ~1.2 ms/core on TRN2
(8 cores in parallel), ~2% above the scheduler cost model's 1.14 ms.
"""

import numpy as np

import concourse.bass as bass
import concourse.mybir as mybir
import concourse.tile as tile

N = 8192
D = 1024
NCORES = 8
M = N // NCORES  # 1024 rows per core
G = 256  # m-group rows
NG = M // G  # 4 groups
P = 128
KO = D // P  # 8 contraction chunks
NCH = 64  # n-chunks of 128 per group sweep
SCALE = 1.0 / 32.0  # 1/sqrt(D)

F32R = mybir.dt.float32r
F32 = mybir.dt.float32
F16 = mybir.dt.float16


def _round_fp32r(x: np.ndarray) -> np.ndarray:
    """Round fp32 to fp32r (keep 11 mantissa bits, RNE) like the PE expects."""
    u = x.view(np.uint32).astype(np.uint64)
    low = u & np.uint64(0xFFF)
    base = u & ~np.uint64(0xFFF)
    round_up = (low > 0x800) | ((low == 0x800) & (((base >> np.uint64(12)) & np.uint64(1)) == 1))
    out = base + np.where(round_up, np.uint64(0x1000), np.uint64(0))
    return out.astype(np.uint32).view(np.float32)


def _split_f16(x: np.ndarray):
    hi = x.astype(np.float16)
    lo = (x - hi.astype(np.float32)).astype(np.float16)
    return hi, lo


def _split_waits(nc, max_waits: int = 1):
    """walrus in this toolchain fits only ~1 embedded sync-wait per
    instruction; hoist extras onto standalone NoOps on the same engine."""
    ctr = 0
    for fn in nc.m.functions:
        for bb in fn.blocks:
            insts = list(bb.instructions)
            out = []
            changed = False
            for inst in insts:
                si = getattr(inst, "sync_info", None)
                waits = list(si.on_wait) if si is not None and si.on_wait else []
                if len(waits) > max_waits:
                    changed = True
                    hoist, keep = waits[:-max_waits], waits[-max_waits:]
                    for i in range(0, len(hoist), max_waits):
                        nop = mybir.InstNoOp(name=f"I-waitsplit-{ctr}")
                        ctr += 1
                        nop.engine = inst.engine
                        nop.sync_info = mybir.SyncInfo(
                            on_wait=hoist[i : i + max_waits], on_update=[]
                        )
                        out.append(nop)
                    inst.sync_info = mybir.SyncInfo(
                        on_wait=keep, on_update=list(si.on_update)
                    )
                out.append(inst)
            if changed:
                bb.instructions = out
    return nc


def _mm3(nc, ps, lhs_hi, lhs_lo, rhs_hi, rhs_lo, k_range, first, last):
    """Accumulate the 3-product hi/lo split into psum `ps`.
    lhs_*/rhs_* are callables k -> AP. Order groups products sharing the
    stationary operand (lhsT) so weight loads amortize."""
    n = len(k_range)
    for i, k in enumerate(k_range):
        nc.tensor.matmul(
            ps, lhs_hi(k), rhs_hi(k), start=(first and i == 0), stop=False
        )
        nc.tensor.matmul(ps, lhs_hi(k), rhs_lo(k), start=False, stop=False)
        nc.tensor.matmul(
            ps, lhs_lo(k), rhs_hi(k), start=False, stop=(last and i == n - 1)
        )


def build_nc(
    split_waits: bool = True,
    reps: int = 1,
    lt_bufs: int = 3,
    xt_bufs: int = 3,
    x_bufs: int = 6,
    e_bufs: int = 4,
    out_bufs: int = 2,
    stat_bufs: int = 2,
    pro_bufs: int = 8,
):
    nc = bass.Bass("TRN2", target_bir_lowering=False)
    x_d = nc.dram_tensor("x", [N, D], F16, kind="ExternalInput").ap()
    xth_d = nc.dram_tensor("xth", [D, N], F16, kind="ExternalInput").ap()
    xtl_d = nc.dram_tensor("xtl", [D, N], F16, kind="ExternalInput").ap()
    rh_d = nc.dram_tensor("rh", [D, D], F16, kind="ExternalInput").ap()
    rl_d = nc.dram_tensor("rl", [D, D], F16, kind="ExternalInput").ap()
    eth_d = nc.dram_tensor("eth", [D, D], F16, kind="ExternalInput").ap()
    etl_d = nc.dram_tensor("etl", [D, D], F16, kind="ExternalInput").ap()
    xith_d = nc.dram_tensor("xith", [D, M], F16, kind="ExternalInput").ap()
    xitl_d = nc.dram_tensor("xitl", [D, M], F16, kind="ExternalInput").ap()
    out_d = nc.dram_tensor("out", [M, D], F32, kind="ExternalOutput").ap()

    def r3(ap):  # [D, W] dram -> [128, KO, W]
        return ap.rearrange("(ko p) w -> p ko w", p=P)

    with tile.TileContext(nc) as tc:
        with (
            tc.tile_pool(name="dram", bufs=1, space="DRAM") as dram_pool,
        ):
            pth_dram = dram_pool.tile([D, M], F16, name="pth_dram")
            ptl_dram = dram_pool.tile([D, M], F16, name="ptl_dram")
            nmrow_dram = dram_pool.tile([NG, G], F32, name="nmrow_dram")

            # ---------------- prologue: QT, PT ----------------
            with (
                tc.tile_pool(name="pro", bufs=1) as pro,
                tc.tile_pool(name="pro_ps", bufs=pro_bufs, space="PSUM") as pro_ps,
            ):
                rh = pro.tile([P, KO, D], F16, name="rh")
                rl = pro.tile([P, KO, D], F16, name="rl")
                eth = pro.tile([P, KO, D], F16, name="eth")
                etl = pro.tile([P, KO, D], F16, name="etl")
                xith = pro.tile([P, KO, M], F16, name="xith")
                xitl = pro.tile([P, KO, M], F16, name="xitl")
                for t, d in (
                    (rh, rh_d), (rl, rl_d), (eth, eth_d), (etl, etl_d),
                    (xith, xith_d), (xitl, xitl_d),
                ):
                    nc.sync.dma_start(t, r3(d))

                qth = pro.tile([P, KO, M], F16, name="qth")
                qtl = pro.tile([P, KO, M], F16, name="qtl")
                # QT[c, m] = sum_d' R[d', c] XiT[d', m]
                for co in range(KO):
                    for mh in range(2):
                        ms = slice(mh * 512, (mh + 1) * 512)
                        ps = pro_ps.tile([P, 512], F32, name="pro_psum")
                        cs = slice(co * P, (co + 1) * P)
                        _mm3(
                            nc, ps,
                            lambda k, cs=cs: rh[:, k, cs],
                            lambda k, cs=cs: rl[:, k, cs],
                            lambda k, ms=ms: xith[:, k, ms],
                            lambda k, ms=ms: xitl[:, k, ms],
                            range(KO), True, True,
                        )
                        nc.scalar.copy(qth[:, co, ms], ps)
                        nc.vector.tensor_tensor(
                            qtl[:, co, ms], ps, qth[:, co, ms],
                            mybir.AluOpType.subtract,
                        )
                pth = pro.tile([P, KO, M], F16, name="pth")
                ptl = pro.tile([P, KO, M], F16, name="ptl")
                # PT[d, m] = sum_c ET[c, d] QT[c, m]
                for do in range(KO):
                    for mh in range(2):
                        ms = slice(mh * 512, (mh + 1) * 512)
                        ps = pro_ps.tile([P, 512], F32, name="pro_psum")
                        ds = slice(do * P, (do + 1) * P)
                        _mm3(
                            nc, ps,
                            lambda k, ds=ds: eth[:, k, ds],
                            lambda k, ds=ds: etl[:, k, ds],
                            lambda k, ms=ms: qth[:, k, ms],
                            lambda k, ms=ms: qtl[:, k, ms],
                            range(KO), True, True,
                        )
                        nc.scalar.copy(pth[:, do, ms], ps)
                        nc.vector.tensor_tensor(
                            ptl[:, do, ms], ps, pth[:, do, ms],
                            mybir.AluOpType.subtract,
                        )
                nc.sync.dma_start(r3(pth_dram[:]), pth)
                nc.sync.dma_start(r3(ptl_dram[:]), ptl)

            # ---------------- main loop over m-groups ----------------
            with (
                tc.tile_pool(name="lbuf", bufs=1) as lpool,
                tc.tile_pool(name="ptg", bufs=2) as ptg_pool,
                tc.tile_pool(name="xts", bufs=xt_bufs) as xt_pool,
                tc.tile_pool(name="xs", bufs=x_bufs) as x_pool,
                tc.tile_pool(name="es", bufs=e_bufs) as e_pool,
                tc.tile_pool(name="outs", bufs=out_bufs) as out_pool,
                tc.tile_pool(name="stats", bufs=stat_bufs) as st_pool,
                tc.tile_pool(name="lt_ps", bufs=lt_bufs, space="PSUM") as lt_ps,
                tc.tile_pool(name="av_ps", bufs=2, space="PSUM") as av_ps,
                tc.tile_pool(name="st_ps", bufs=1, space="PSUM") as st_ps,
            ):
                l_sb = lpool.tile([P, NCH, G], F32, name="l_sb")
                ones1 = lpool.tile([1, P], F32, name="ones1")
                nc.vector.memset(ones1, 1.0)
                ident = lpool.tile([P, P], F32, name="ident")
                from concourse.masks import make_identity

                make_identity(nc, ident)

                def emit_group(g):
                    gs = slice(g * G, (g + 1) * G)
                    ptgh = ptg_pool.tile([P, KO, G], F16, name="ptgh")
                    ptgl = ptg_pool.tile([P, KO, G], F16, name="ptgl")
                    nc.sync.dma_start(ptgh, r3(pth_dram[:])[:, :, gs])
                    nc.sync.dma_start(ptgl, r3(ptl_dram[:])[:, :, gs])
                    rm = st_pool.tile([P, G], F32, name="rm")
                    # --- LT: 16 stream tiles x 4 n-subchunks ---
                    for jt in range(16):
                        ns = slice(jt * 512, (jt + 1) * 512)
                        xth_t = xt_pool.tile([P, KO, 512], F16, name="xth_t")
                        xtl_t = xt_pool.tile([P, KO, 512], F16, name="xtl_t")
                        nc.sync.dma_start(xth_t, r3(xth_d)[:, :, ns])
                        nc.sync.dma_start(xtl_t, r3(xtl_d)[:, :, ns])
                        for js in range(4):
                            j = jt * 4 + js
                            sl = slice(js * P, (js + 1) * P)
                            ps = lt_ps.tile([P, G], F32, name="lt_psum")
                            _mm3(
                                nc, ps,
                                lambda k, sl=sl: xth_t[:, k, sl],
                                lambda k, sl=sl: xtl_t[:, k, sl],
                                lambda k: ptgh[:, k],
                                lambda k: ptgl[:, k],
                                range(KO), True, True,
                            )
                            nc.scalar.copy(l_sb[:, j], ps)
                            if j == 0:
                                nc.vector.tensor_copy(rm, l_sb[:, 0])
                            else:
                                nc.vector.tensor_tensor(
                                    rm, rm, l_sb[:, j], mybir.AluOpType.max
                                )
                    # --- stats: column max over partitions via PE transpose,
                    # negate, DRAM-bounce to a row, broadcast back via K=1 mm
                    nmcol = st_pool.tile([P, 2], F32, name="nmcol")
                    for mt in range(2):
                        t_ps = st_ps.tile([P, G], F32, name="t_ps", tag="stps")[:, :P]
                        nc.tensor.transpose(
                            t_ps, rm[:, mt * P : (mt + 1) * P], ident
                        )
                        nc.vector.tensor_reduce(
                            nmcol[:, mt : mt + 1], t_ps,
                            axis=mybir.AxisListType.X, op=mybir.AluOpType.max,
                            negate=True,
                        )
                    nc.sync.dma_start(
                        nmrow_dram[g].rearrange("(t p) -> p t", p=P), nmcol
                    )
                    negrow = st_pool.tile([1, G], F32, name="negrow")
                    nc.sync.dma_start(
                        negrow, nmrow_dram[g].rearrange("(o w) -> o w", o=1)
                    )
                    nm_ps = st_ps.tile([P, G], F32, name="nm_ps", tag="stps")
                    nc.tensor.matmul(nm_ps, ones1, negrow, start=True, stop=True)
                    negmax = st_pool.tile([P, G], F32, name="negmax")
                    nc.scalar.copy(negmax, nm_ps)
                    # --- exp + AV ---
                    av_acc = [
                        av_ps.tile([P, D], F32, name="av_psum") for _ in range(2)
                    ]
                    sacc = st_pool.tile([P, G], F32, name="sacc")
                    for jt in range(16):
                        for js in range(4):
                            j = jt * 4 + js
                            x_t = x_pool.tile([P, D], F16, name="x_t")
                            nc.sync.dma_start(
                                x_t, x_d[j * P : (j + 1) * P, :]
                            )
                            tmp = e_pool.tile([P, G], F32, name="tmp")
                            e_t = e_pool.tile([P, G], F16, name="e_t")
                            nc.vector.tensor_tensor(
                                tmp, l_sb[:, j], negmax, mybir.AluOpType.add
                            )
                            nc.scalar.activation(
                                e_t, tmp, mybir.ActivationFunctionType.Exp,
                                scale=SCALE,
                            )
                            if j == 0:
                                nc.vector.tensor_copy(sacc, e_t)
                            else:
                                nc.vector.tensor_tensor(
                                    sacc, sacc, e_t, mybir.AluOpType.add
                                )
                            for mt in range(2):
                                for dh in range(2):
                                    nc.tensor.matmul(
                                        av_acc[mt][:, dh * 512 : (dh + 1) * 512],
                                        e_t[:, mt * P : (mt + 1) * P],
                                        x_t[:, dh * 512 : (dh + 1) * 512],
                                        start=(j == 0),
                                        stop=(j == NCH - 1),
                                    )
                    # --- sums -> per-m-tile reciprocal; finalize ---
                    scol = st_pool.tile([P, 2], F32, name="scol")
                    for mt in range(2):
                        t_ps = st_ps.tile([P, G], F32, name="t_ps", tag="stps")[:, :P]
                        nc.tensor.transpose(
                            t_ps, sacc[:, mt * P : (mt + 1) * P], ident
                        )
                        nc.vector.tensor_reduce(
                            scol[:, mt : mt + 1], t_ps,
                            axis=mybir.AxisListType.X, op=mybir.AluOpType.add,
                        )
                    rcol = st_pool.tile([P, 2], F32, name="rcol")
                    nc.vector.reciprocal(rcol, scol)
                    for mt in range(2):
                        o_sb = out_pool.tile([P, D], F32, name="o_sb")
                        nc.vector.tensor_scalar_mul(
                            o_sb, av_acc[mt], rcol[:, mt : mt + 1]
                        )
                        row0 = g * G + mt * P
                        nc.sync.dma_start(out_d[row0 : row0 + P], o_sb)

                if reps == 1:
                    for g in range(NG):
                        emit_group(g)
                else:
                    with tc.For_i(0, reps, 1):
                        for g in range(NG):
                            emit_group(g)

    if split_waits:
        _split_waits(nc)
    return nc


_CACHE = {}


def _prep_inputs(x, rot, ent):
    x_r = np.ascontiguousarray(x).astype(np.float16)
    xt = np.ascontiguousarray(x.T)
    xth, xtl = _split_f16(xt)
    rhh, rll = _split_f16(rot)
    et = np.ascontiguousarray(ent.T)
    eth, etl = _split_f16(et)
    return x_r, xth, xtl, rhh, rll, eth, etl


def kernel(**inputs) -> np.ndarray:
    from concourse.bass_utils import run_bass_kernel_spmd

    x = np.asarray(inputs["inputs"], dtype=np.float32)
    rot = np.asarray(inputs["rotation"], dtype=np.float32)
    ent = np.asarray(inputs["entangle"], dtype=np.float32)

    x_r, xth, xtl, rhh, rll, eth, etl = _prep_inputs(x, rot, ent)

    if "nc" not in _CACHE:
        _CACHE["nc"] = build_nc()
    nc = _CACHE["nc"]

    in_maps = []
    for c in range(NCORES):
        cs = slice(c * M, (c + 1) * M)
        in_maps.append(
            {
                "x": x_r,
                "xth": xth,
                "xtl": xtl,
                "rh": rhh,
                "rl": rll,
                "eth": eth,
                "etl": etl,
                "xith": np.ascontiguousarray(xth[:, cs]),
                "xitl": np.ascontiguousarray(xtl[:, cs]),
            }
        )
    res = run_bass_kernel_spmd(nc, in_maps, core_ids=list(range(NCORES)))
    out = np.concatenate([res.results[c]["out"] for c in range(NCORES)], axis=0)
    return np.ascontiguousarray(out.astype(np.float32))


if __name__ == "__main__":
    rng = np.random.default_rng(0)
    x = rng.standard_normal((N, D)).astype(np.float32)
    r = rng.standard_normal((D, D)).astype(np.float32)
    e = rng.standard_normal((D, D)).astype(np.float32)
    o = kernel(inputs=x, rotation=r, entangle=e)
    print(o.shape, o.dtype, float(np.abs(o).max()))

